# revision 12
# baseline (speedup 1.0000x reference)
"""Cross-attention kernel for TRN2, SPMD over 8 NeuronCores.

Problem: B=8, SQ=4096, SKV=77, D_EMBED=1024, D_CROSS=768, H=16, DH=64.
  q = x @ wq + bq ; k = y @ wk + bk ; v = y @ wv + bv
  out = softmax(q k^T / 8) v @ wo + bo

Sharding: pure data-parallel over batch (1 batch element per core, no
collectives). Host pre-transposes x and y per core so the device kernel
keeps every tensor feature-major (contraction dim on partitions) until the
O-projection, which uses attnout^T as the stationary operand to emit the
output in natural row-major layout.

Compute dtype: bf16 operands (host-cast), fp32 PSUM accumulation, fp32 out.

Schedule: two-stage software pipeline over 512-wide query chunks. Iteration
i emits pass A for chunk i (Q-projection with score matmuls and exps
trailing one column-tile group behind) and pass B for chunk i-1 (softmax
normalization broadcast, PV, O-projection). The exp->sum->reciprocal chain
for chunk i completes while pass B work keeps the PE busy, so the tensor
engine never idles long enough for the HAM clock gate to re-throttle.

Engine placement per iteration (j = i-1):
  PE : sums(j) | {Qproj ct, scores pair}x8 (i) | {rb, PV}x4 pairs (j) | Oproj (j)
  ACT: exps paired 2-heads/inst (i), rb pair copies (j)
  DVE: reciprocal_approx_fast(j), qT copies+bias (i), aoT muls (j), out bias adds (j)

Head-paired PSUM tiles ([*, 2, 512] spanning two adjacent banks) let one
ACT instruction drain two heads' scores (exp) or two broadcast tiles,
halving ACT instruction count; scores pairs and rb pairs share a 4-bank
PSUM pool so the broadcast matmuls are not gated on prompt ACT drains.

Softmax is computed without max-subtraction (scores are O(5) for this
problem class; exp stays comfortably inside fp32/bf16 range):
  scoresT[s,q] = k'_h @ q_h^T with k' = (k + bk)/8 folded at k-projection
  e = exp(scoresT)  (bf16)
  r = 1 / (sel16^T @ e)         per-head [16, SQ] sums via PE column-sum
  aoT[d,q] = (v_h^T @ e) * bcast(r)   (normalization commutes)
  out[q,:] = aoT^T @ wo + bo    (bo added during the PSUM drain)
"""

import numpy as np
import ml_dtypes

import concourse.bass as bass
import concourse.mybir as mybir
import concourse.tile as tile
from concourse import bacc
from concourse import bass_utils

F32 = mybir.dt.float32
BF16 = mybir.dt.bfloat16
AF = mybir.ActivationFunctionType

B = 8
SQ = 4096
SKV = 77
D = 1024
DC = 768
H = 16
DH = 64
KT = D // 128    # 8 embed k-tiles
KC = DC // 128   # 6 cross k-tiles
CT = D // 128    # 8 column tiles of the 1024-wide projections
CH = 512         # query chunk
NCH = SQ // CH   # 8 chunks
NQT = CH // 128  # 4 query 128-tiles per chunk
HP = H // 2      # 8 head pairs

_CACHED = {}


def _build():
    nc = bacc.Bacc("TRN2", target_bir_lowering=False, debug=False, num_devices=B)

    xt = nc.dram_tensor("xt", (D, SQ), BF16, kind="ExternalInput")
    yt = nc.dram_tensor("yt", (DC, SKV), BF16, kind="ExternalInput")
    wq_d = nc.dram_tensor("wq", (D, D), BF16, kind="ExternalInput")
    wk_d = nc.dram_tensor("wk", (DC, D), BF16, kind="ExternalInput")
    wv_d = nc.dram_tensor("wv", (DC, D), BF16, kind="ExternalInput")
    wo_d = nc.dram_tensor("wo", (D, D), BF16, kind="ExternalInput")
    bq32_d = nc.dram_tensor("bq32", (1, D), F32, kind="ExternalInput")
    bk8_d = nc.dram_tensor("bk8", (1, D), F32, kind="ExternalInput")
    bv_d = nc.dram_tensor("bv", (1, D), BF16, kind="ExternalInput")
    bob_d = nc.dram_tensor("bob", (128, D), F32, kind="ExternalInput")
    oneh_d = nc.dram_tensor("oneh", (SKV, 2 * 128), BF16, kind="ExternalInput")
    out_d = nc.dram_tensor("out", (SQ, D), F32, kind="ExternalOutput")

    with tile.TileContext(nc) as tc:
        with (
            tc.tile_pool(name="consts", bufs=1) as consts,
            tc.tile_pool(name="wpool", bufs=1) as wpool,
            tc.tile_pool(name="xpool", bufs=2) as xpool,
            tc.tile_pool(name="qpool", bufs=2) as qpool,
            tc.tile_pool(name="epool", bufs=2) as epool,
            tc.tile_pool(name="rbpool", bufs=2) as rbpool,
            tc.tile_pool(name="aopool", bufs=2) as aopool,
            tc.tile_pool(name="opool", bufs=3) as opool,
            tc.tile_pool(name="pq", bufs=2, space="PSUM") as pq,
            tc.tile_pool(name="psc", bufs=2, space="PSUM") as psc,
            tc.tile_pool(name="ppv", bufs=2, space="PSUM") as ppv,
        ):
            # ---- constants / weights, DMA'd in first-use order ----
            yt_sb = consts.tile([128, KC, SKV], BF16, tag="yt")
            nc.scalar.dma_start(yt_sb[:], yt.ap().rearrange("(kt p) s -> p kt s", p=128))
            bk8_sb = consts.tile([128, CT], F32, tag="bk8")
            nc.scalar.dma_start(bk8_sb[:], bk8_d.ap().rearrange("a (ct p) -> (a p) ct", p=128))
            bq32_sb = consts.tile([128, CT], F32, tag="bq32")
            nc.scalar.dma_start(bq32_sb[:], bq32_d.ap().rearrange("a (ct p) -> (a p) ct", p=128))

            xT = [None] * NCH
            def dma_x(c):
                xT[c] = xpool.tile([128, KT, CH], BF16, tag="xT", name="xT")
                nc.sync.dma_start(
                    xT[c][:],
                    xt.ap().rearrange("(kt p) q -> p kt q", p=128)[:, :, c * CH:(c + 1) * CH],
                )
            dma_x(0)

            # wq/wk streamed in 128-column granules, interleaved in the
            # order iteration 0 consumes them, so Qproj(0) starts as soon as
            # xT0 + the first granule land instead of after the full 4.6MB
            wq_sb = wpool.tile([128, KT, D], BF16, tag="wq")
            wk_sb = wpool.tile([128, KC, D], BF16, tag="wk")
            wqr = wq_d.ap().rearrange("(kt p) n -> p kt n", p=128)
            wkr = wk_d.ap().rearrange("(kt p) n -> p kt n", p=128)
            def dma_wslice(w_sb, w_ap, g):
                nc.sync.dma_start(
                    w_sb[:, :, g * 128:(g + 1) * 128], w_ap[:, :, g * 128:(g + 1) * 128]
                )
            dma_wslice(wk_sb, wkr, 0)
            dma_wslice(wk_sb, wkr, 1)
            for g in range(CT):
                dma_wslice(wq_sb, wqr, g)
                if g in (0, 1, 2):
                    dma_wslice(wk_sb, wkr, 2 * g + 2)
                    dma_wslice(wk_sb, wkr, 2 * g + 3)
                if g == 5:
                    dma_x(1)

            wv_sb = wpool.tile([128, KC, D], BF16, tag="wv")
            nc.scalar.dma_start(wv_sb[:], wv_d.ap().rearrange("(kt p) n -> p kt n", p=128))
            bv_sb = consts.tile([1, D], BF16, tag="bv")
            nc.scalar.dma_start(bv_sb[:], bv_d.ap())
            oneh_sb = consts.tile([SKV, 2, 128], BF16, tag="oneh")
            nc.scalar.dma_start(oneh_sb[:], oneh_d.ap().rearrange("s (a p) -> s a p", p=128))

            wo_sb = wpool.tile([128, KT, D], BF16, tag="wo")
            nc.scalar.dma_start(wo_sb[:], wo_d.ap().rearrange("(kt p) n -> p kt n", p=128))
            bob_sb = consts.tile([128, D], F32, tag="bob")
            nc.scalar.dma_start(bob_sb[:], bob_d.ap())

            ones77r = consts.tile([1, SKV], BF16, tag="ones77r")
            nc.vector.memset(ones77r[:], 1.0)

            kT_sb = consts.tile([128, CT, SKV], BF16, tag="kT")
            v_aug = consts.tile([SKV, H, DH], BF16, tag="v")

            # k projection: kT[c, s] = sum_k wk[k, c] yT[k, s]; fold (.+bk)/8.
            # Emitted inside iteration 0 between Q-projection groups so the
            # in-order PE starts on Qproj as soon as xT0+wq land.
            def k_proj(ct0, ct1):
                for ct in range(ct0, ct1):
                    psk = pq.tile([128, CH], F32, tag="mm", name="psk")
                    for kt in range(KC):
                        nc.tensor.matmul(
                            psk[:, 0:SKV],
                            wk_sb[:, kt, ct * 128:(ct + 1) * 128],
                            yt_sb[:, kt, :],
                            start=(kt == 0),
                            stop=(kt == KC - 1),
                        )
                    nc.scalar.activation(
                        kT_sb[:, ct, :],
                        psk[:, 0:SKV],
                        AF.Identity,
                        scale=0.125,
                        bias=bk8_sb[:, ct:ct + 1],
                    )

            # v projection body: emitted inside iteration 0 so the in-order
            # PE is not blocked on the wv DMA before Qproj(0) can start
            def v_proj():
                for n in range(2):
                    psv = pq.tile([128, CH], F32, tag="mm", name="psv")
                    for kt in range(KC):
                        nc.tensor.matmul(
                            psv[0:SKV, :],
                            yt_sb[:, kt, :],
                            wv_sb[:, kt, n * 512:(n + 1) * 512],
                            start=(kt == 0),
                            stop=False,
                        )
                    nc.tensor.matmul(
                        psv[0:SKV, :],
                        ones77r[:],
                        bv_sb[0:1, n * 512:(n + 1) * 512],
                        start=False,
                        stop=True,
                    )
                    for j in range(8):
                        h = n * 8 + j
                        nc.any.tensor_copy(v_aug[:, h, :], psv[0:SKV, j * DH:(j + 1) * DH])

            k_proj(0, 2)

            # ---- software-pipelined main loop ----
            qT = [None] * NCH
            e_ch = [None] * NCH

            for i in range(NCH + 1):
                j = i - 1  # pass-B chunk

                if i + 2 < NCH:
                    dma_x(i + 2)

                rb_sbs = [None] * (HP // 2)
                if j >= 0:
                    # per-head exp-sums of chunk j, broadcast across all 128
                    # partitions arranged per head-pair, then reciprocal
                    for hpp in range(HP // 2):
                        sb_ps = psc.tile([128, 2, CH], F32, tag="sc", name="sb_ps")
                        for half in range(2):
                            hp = 2 * hpp + half
                            for h2 in range(2):
                                h = 2 * hp + h2
                                nc.tensor.matmul(
                                    sb_ps[:, half, :],
                                    oneh_sb[:, h2, :],
                                    e_ch[j][:, h, :],
                                    start=(h2 == 0),
                                    stop=(h2 == 1),
                                )
                        rb_sbs[hpp] = rbpool.tile(
                            [128, 2, CH], F32, tag="rb_sb", name="rb_sb", bufs=4,
                        )
                        nc.vector.reciprocal_approx_fast(rb_sbs[hpp][:], sb_ps[:])

                if i < NCH:
                    # pass A: q^T projection; paired scores+exp trail one ct-group
                    qT[i] = qpool.tile([128, CT, CH], BF16, tag="qT", name="qT")
                    e_ch[i] = epool.tile([SKV, H, CH], BF16, tag="e", name="e_ch")

                    def scores_pair(g):
                        pssc = psc.tile([SKV, 2, CH], F32, tag="sc", name="pssc")
                        for half in range(2):
                            h = 2 * g + half
                            nc.tensor.matmul(
                                pssc[:, half, :],
                                kT_sb[(h % 2) * 64:(h % 2) * 64 + 64, h // 2, :],
                                qT[i][(h % 2) * 64:(h % 2) * 64 + 64, h // 2, :],
                                start=True, stop=True,
                            )
                        nc.scalar.activation(
                            e_ch[i][:, 2 * g:2 * g + 2, :], pssc[:], AF.Exp,
                        )

                    for g in range(CT):
                        psq = pq.tile([128, CH], F32, tag="mm", name="psq")
                        for kt in range(KT):
                            nc.tensor.matmul(
                                psq[:],
                                wq_sb[:, kt, g * 128:(g + 1) * 128],
                                xT[i][:, kt, :],
                                start=(kt == 0),
                                stop=(kt == KT - 1),
                            )
                        nc.vector.tensor_scalar_add(
                            qT[i][:, g, :], psq[:], bq32_sb[:, g:g + 1],
                        )
                        if i == 0 and g < 3:
                            k_proj(2 * g + 2, 2 * g + 4)
                        if g >= 1:
                            scores_pair(g - 1)
                    scores_pair(CT - 1)
                    if i == 1:
                        v_proj()

                if j >= 0:
                    # pass B: PV pairs, aoT = PV * bcast(r)
                    aoT = aopool.tile([128, KT, CH], BF16, tag="aoT", name="aoT")
                    for hpp in range(HP // 2):
                        for half in range(2):
                            hp = 2 * hpp + half
                            pspv = ppv.tile([128, CH], F32, tag="pv", name="pspv")
                            for h2 in range(2):
                                h = 2 * hp + h2
                                nc.tensor.matmul(
                                    pspv[h2 * 64:(h2 + 1) * 64, :],
                                    v_aug[:, h, :],
                                    e_ch[j][:, h, :],
                                    start=True, stop=True,
                                )
                            nc.vector.tensor_mul(aoT[:, hp, :], pspv[:], rb_sbs[hpp][:, half, :])

                    # O-projection with bias added during the PSUM drain
                    q0 = j * CH
                    for qt in range(NQT):
                        for n in range(2):
                            pso = pq.tile([128, CH], F32, tag="mm", name="pso")
                            for kt in range(KT):
                                nc.tensor.matmul(
                                    pso[:],
                                    aoT[:, kt, qt * 128:(qt + 1) * 128],
                                    wo_sb[:, kt, n * 512:(n + 1) * 512],
                                    start=(kt == 0),
                                    stop=(kt == KT - 1),
                                )
                            o_sb = opool.tile([128, 512], F32, tag="o")
                            nc.vector.tensor_add(o_sb[:], pso[:], bob_sb[:, n * 512:(n + 1) * 512])
                            nc.scalar.dma_start(
                                out_d.ap()[q0 + qt * 128: q0 + (qt + 1) * 128,
                                           n * 512:(n + 1) * 512],
                                o_sb[:],
                            )

    nc.compile()
    return nc


def _get_nc():
    if "nc" not in _CACHED:
        _CACHED["nc"] = _build()
    return _CACHED["nc"]


def _prep_in_maps(x, y, wq, bq, wk, bk, wv, bv, wo, bo):
    x = np.asarray(x)
    y = np.asarray(y)
    bf = ml_dtypes.bfloat16
    wq_b = np.asarray(wq).astype(bf)
    wk_b = np.asarray(wk).astype(bf)
    wv_b = np.asarray(wv).astype(bf)
    wo_b = np.asarray(wo).astype(bf)
    bq32 = np.asarray(bq).reshape(1, D).astype(np.float32)
    bv_b = np.asarray(bv).reshape(1, D).astype(bf)
    bob = np.broadcast_to(np.asarray(bo).reshape(1, D).astype(np.float32), (128, D)).copy()
    bk8 = (np.asarray(bk).reshape(1, D) * 0.125).astype(np.float32)
    oneh = np.zeros((SKV, 2, 128), np.float32)
    oneh[:, 0, 0:64] = 1.0
    oneh[:, 1, 64:128] = 1.0
    oneh = oneh.reshape(SKV, 2 * 128).astype(bf)

    in_maps = []
    for b in range(B):
        in_maps.append({
            "xt": np.ascontiguousarray(x[b].T).astype(bf),
            "yt": np.ascontiguousarray(y[b].T).astype(bf),
            "wq": wq_b, "wk": wk_b, "wv": wv_b, "wo": wo_b,
            "bq32": bq32, "bk8": bk8, "bv": bv_b, "bob": bob,
            "oneh": oneh,
        })
    return in_maps


def kernel(x, y, wq, bq, wk, bk, wv, bv, wo, bo):
    in_maps = _prep_in_maps(x, y, wq, bq, wk, bk, wv, bv, wo, bo)
    nc = _get_nc()
    res = bass_utils.run_bass_kernel_spmd(nc, in_maps, core_ids=list(range(B)))
    out = np.stack([res.results[b]["out"] for b in range(B)], axis=0)
    return out.astype(np.float32)
